# revision 1
# baseline (speedup 1.0000x reference)
"""AxialAttention (width=False, no positional) on 8 Trainium2 NeuronCores.

Sharding: data-parallel over N (8 images -> 8 cores, one image each);
all conv/BN params replicated. Each core runs the full per-image axial
attention (rows attend independently along H for each w-column).

Hardcoded problem shape: x (8, 128, 128, 128) f32, w_qkv (256, 128),
groups=8, out_planes=128.
"""

import numpy as np
import jax
import jax.numpy as jnp

EPS = 1e-5
GROUPS = 8


def _bn(x, gamma, beta, mean, var, axis):
    shape = [1] * x.ndim
    shape[axis] = -1
    scale = gamma.reshape(shape) * jax.lax.rsqrt(var.reshape(shape) + EPS)
    return (x - mean.reshape(shape)) * scale + beta.reshape(shape)


def _axial_one_image(x, w_qkv, qkv_gamma, qkv_beta, qkv_mean, qkv_var,
                     sim_gamma, sim_beta, sim_mean, sim_var,
                     out_gamma, out_beta, out_mean, out_var):
    # x: (C, H, W) one image
    C, H, W = x.shape
    out_planes = w_qkv.shape[0] // 2
    gp = out_planes // GROUPS

    # (C, H, W) -> (W, C, H)
    xb = jnp.transpose(x, (2, 0, 1))
    qkv = jnp.einsum('bci,oc->boi', xb, w_qkv)
    qkv = _bn(qkv, qkv_gamma, qkv_beta, qkv_mean, qkv_var, axis=1)
    qkv = qkv.reshape(W, GROUPS, 2 * gp, H)
    q = qkv[:, :, : gp // 2]
    k = qkv[:, :, gp // 2: gp]
    v = qkv[:, :, gp:]

    qk = jnp.einsum('bgci,bgcj->bgij', q, k)
    sim = _bn(qk, sim_gamma, sim_beta, sim_mean, sim_var, axis=1)
    sim = jax.nn.softmax(sim, axis=3)

    sv = jnp.einsum('bgij,bgcj->bgci', sim, v)  # (W, g, gp, H)
    sv = sv.reshape(W, out_planes, H)
    out = _bn(sv, out_gamma, out_beta, out_mean, out_var, axis=1)
    # (W, out_planes, H) -> (out_planes, H, W)
    return jnp.transpose(out, (1, 2, 0))


_pmapped = None


def _get_pmapped():
    global _pmapped
    if _pmapped is None:
        _pmapped = jax.pmap(
            _axial_one_image,
            in_axes=(0,) + (None,) * 13,
            devices=jax.devices()[:8],
        )
    return _pmapped


def kernel(x, w_qkv, qkv_gamma, qkv_beta, qkv_mean, qkv_var,
           sim_gamma, sim_beta, sim_mean, sim_var,
           out_gamma, out_beta, out_mean, out_var):
    f = _get_pmapped()
    out = f(jnp.asarray(x, jnp.float32), w_qkv, qkv_gamma, qkv_beta,
            qkv_mean, qkv_var, sim_gamma, sim_beta, sim_mean, sim_var,
            out_gamma, out_beta, out_mean, out_var)
    # (N=8, out_planes, H, W) == full output
    return np.asarray(jax.device_get(out), dtype=np.float32)
